# revision 1
# baseline (speedup 1.0000x reference)
"""Trainium2 Bass kernel for InterventionAwareStructure loss.

loss = sum_b,i,d A[b,i,d] * mask[regimes[b], d] / count   (scalar)

Data-parallel over batch across 8 NeuronCores. Each core:
  - streams its A shard [32, 512, 512] from HBM on the SP HWDGE ring
    (fp32 bits re-tagged as fp32r via a dram-tensor bitcast, so no
    SWDGE cast path): 7 chunks of 4 MB with 32 KB descriptor lines at
    SDMA line rate, then the last four batch items as 1 MB chunks so
    the tail matmul backlog stays short,
  - TensorE reduces each chunk over the source axis i with one-hot
    block stationaries (1 cycle/row in fp32r), accumulating ALL 128
    matmuls into a single [32, 512] PSUM tile.  The stationary table
    is synthesized on the otherwise-idle GpSimd engine
    (memset + affine_select), so no weight bytes ride the HBM stream,
  - one final VectorE copy moves the PSUM colsums to SBUF and they are
    DMA'd out as [32, 512] on the idle ACT HWDGE ring; the host does
    the tiny mask dot, the cross-core sum, and the divide by count.

The mask gather (256x512), the mask dot, and the final scalar
reduction are all done on host; they are negligible next to the
256 MB stream of A.
"""

import numpy as np

import concourse.bass as bass
import concourse.tile as tile
from concourse import bacc, mybir
from concourse.bass_utils import run_bass_kernel_spmd

INTERVENTION_STRENGTH = 1.0

N_CORES = 8
B, N_REGIMES, D = 256, 16, 512
B_SH = B // N_CORES          # 32 batch items per core
NBIG = B_SH // 4 - 1         # 7 full 4 MB chunks (4 batch items each)
FREE4 = 4 * D * D // 128     # 8192 f32 per partition per 4 MB chunk
FREE1 = D * D // 128         # 2048 f32 per partition per 1 MB chunk
NTAIL = 4                    # last 4 batch items ride 1 MB chunks
WCOLS = (NBIG + NTAIL) * 32  # stationary table

_CACHED_NC = None


def _build_nc() -> bass.Bass:
    nc = bacc.Bacc()
    f32 = mybir.dt.float32
    f32r = mybir.dt.float32r

    # fp32 bits, tagged fp32r so HWDGE can move them without a cast.
    a = nc.dram_tensor("a", [B_SH, D, D], f32, kind="ExternalInput").bitcast(f32r)
    out = nc.dram_tensor("out", [B_SH, D], f32, kind="ExternalOutput")

    # 4 MB chunk g of batches (4g..4g+3) -> SBUF [128, FREE4]: partition
    # p = (gb * 32 + ih) holds rows i = ih*16 + il of batch 4g+gb; free
    # axis = (il, d) with 32 KB contiguous per partition line.
    a_view4 = a.rearrange(
        "(ng gb) (ih il) d -> ng (gb ih) (il d)", ng=B_SH // 4, ih=32
    )
    # 1 MB tail chunk of batch b -> SBUF [128, FREE1]: partition ih
    # holds rows i = ih*4 + il; free axis = (il, d).
    a_view1 = a.rearrange("b (ih il) d -> b ih (il d)", ih=128)

    with tile.TileContext(nc) as tc:
        with (
            tc.tile_pool(name="big", bufs=4) as big_pool,
            tc.tile_pool(name="tail", bufs=4) as tail_pool,
            tc.tile_pool(name="small", bufs=1) as small_pool,
            tc.tile_pool(name="psum", bufs=2, space="PSUM") as psum_pool,
        ):
            # One-hot block stationary table, built on-chip.  4 MB chunk
            # g uses block g: W[p, g*32 + q] = 1 iff q == 4g + p//32,
            # i.e. per partition quarter (u = g, v = q): v - 4u - gb == 0.
            # The four 1 MB tail chunks use blocks 7..10: one-hot at
            # column 28 + t on all partitions: (u = t, v): v - u - 28 == 0.
            w_f = small_pool.tile([128, WCOLS], f32)
            nc.gpsimd.memset(w_f[:], 1.0)
            for gb in range(4):
                nc.gpsimd.affine_select(
                    out=w_f[gb * 32:(gb + 1) * 32, :NBIG * 32],
                    in_=w_f[gb * 32:(gb + 1) * 32, :NBIG * 32],
                    pattern=[[-4, NBIG], [1, 32]],
                    compare_op=mybir.AluOpType.is_equal,
                    fill=0.0,
                    base=-gb,
                    channel_multiplier=0,
                )
            nc.gpsimd.affine_select(
                out=w_f[:, NBIG * 32:],
                in_=w_f[:, NBIG * 32:],
                pattern=[[-1, NTAIL], [1, 32]],
                compare_op=mybir.AluOpType.is_equal,
                fill=0.0,
                base=-28,
                channel_multiplier=0,
            )
            # The BIR verifier wants fp32r matmul weights produced by an
            # op whose output dtype is fp32r; a DVE copy does the re-tag.
            w_t = small_pool.tile([128, WCOLS], f32r)
            nc.vector.tensor_copy(w_t[:], w_f[:])

            big_tiles = []
            for g in range(NBIG):
                a_t = big_pool.tile([128, FREE4], f32r, tag="a")
                nc.sync.dma_start(a_t[:], a_view4[g])
                big_tiles.append(a_t)
            tail_tiles = []
            for t in range(NTAIL):
                a_t = tail_pool.tile([128, FREE1], f32r, tag="t")
                if t == NTAIL - 1:
                    # Split the final 1 MB into quarters: each 256 KB
                    # piece feeds exactly one matmul, so only ~0.35 us
                    # of PE work trails the last byte of the stream.
                    q4 = FREE1 // 4
                    for q in range(4):
                        nc.sync.dma_start(
                            a_t[:, q * q4:(q + 1) * q4],
                            a_view1[28 + t][:, q * q4:(q + 1) * q4],
                        )
                else:
                    nc.sync.dma_start(a_t[:], a_view1[28 + t])
                tail_tiles.append(a_t)

            # Two PSUM banks: batches 0-27 (the 4 MB chunks) close early
            # so their colsums stream out while the tail chunks are
            # still in flight; only a [4, 512] copy + 8 KB write remain
            # after the last matmul.
            ps_a = psum_pool.tile([B_SH, D], f32, tag="psa")
            for g in range(NBIG):
                a_t = big_tiles[g]
                w_g = w_t[:, g * 32:(g + 1) * 32]
                for j in range(FREE4 // D):
                    nc.tensor.matmul(
                        ps_a[:], w_g, a_t[:, j * D:(j + 1) * D],
                        start=(g == 0 and j == 0),
                        stop=(g == NBIG - 1 and j == FREE4 // D - 1),
                    )
            ps_b = psum_pool.tile([B_SH, D], f32, tag="psb")
            for t in range(NTAIL):
                a_t = tail_tiles[t]
                w_g = w_t[:, (NBIG + t) * 32:(NBIG + t + 1) * 32]
                for j in range(FREE1 // D):
                    nc.tensor.matmul(
                        ps_b[:], w_g, a_t[:, j * D:(j + 1) * D],
                        start=(t == 0 and j == 0),
                        stop=(t == NTAIL - 1 and j == FREE1 // D - 1),
                    )

            nbat = 4 * NBIG
            o_a = small_pool.tile([nbat, D], f32)
            nc.vector.tensor_copy(o_a[:], ps_a[:nbat, :])
            nc.scalar.dma_start(out[:nbat, :], o_a[:])
            # PSUM reads must start at a 32-partition boundary, so copy
            # the full bank and write out only the tail rows.
            o_b = small_pool.tile([B_SH, D], f32)
            nc.vector.tensor_copy(o_b[:], ps_b[:])
            nc.scalar.dma_start(out[nbat:, :], o_b[nbat:, :])

    nc.finalize()
    return nc


def _get_nc() -> bass.Bass:
    global _CACHED_NC
    if _CACHED_NC is None:
        _CACHED_NC = _build_nc()
    return _CACHED_NC


def _run(a_shards, **run_kwargs):
    nc = _get_nc()
    in_maps = [{"a": np.ascontiguousarray(a_shards[c])} for c in range(N_CORES)]
    return run_bass_kernel_spmd(nc, in_maps, list(range(N_CORES)), **run_kwargs)


def kernel(A_per_env, intervention_mask, regimes, _run_kwargs=None):
    A_per_env = np.asarray(A_per_env, dtype=np.float32)
    intervention_mask = np.asarray(intervention_mask, dtype=np.float32)
    regs = np.asarray(regimes).astype(np.int64)

    n_regimes = intervention_mask.shape[0]
    valid = regs < n_regimes
    e = np.clip(regs, 0, n_regimes - 1)
    masks = intervention_mask[e] * valid[:, None].astype(np.float32)  # [B, D]

    a_shards = [A_per_env[c * B_SH:(c + 1) * B_SH] for c in range(N_CORES)]

    res = _run(a_shards, **(_run_kwargs or {}))
    num = np.float64(0.0)
    for c in range(N_CORES):
        colsums = res.results[c]["out"].astype(np.float64)        # [32, 512]
        num += (colsums * masks[c * B_SH:(c + 1) * B_SH]).sum()

    count = masks.astype(np.float64).sum()
    loss = num / count if count > 0 else num
    out = np.asarray(INTERVENTION_STRENGTH * loss, dtype=np.float32)
    if _run_kwargs is not None:
        return out, res
    return out

